# revision 1
# baseline (speedup 1.0000x reference)
"""GCN layer (SpMM) Bass kernel for 8 trn2 NeuronCores.

out[i] = sum_{e: rows[e]==i} edge_vals[e] * embeds[cols[e]]
N=100000 nodes, E=1000000 edges, D=64 features.

Strategy: host sorts edges by destination row and splits nodes into 8
contiguous ranges (12500 nodes/core) with disjoint outputs -> no
collectives. Per core, output rows are processed in blocks of 128; each
block's edges are padded to chunks of 128. Per chunk the device does:
  1. indirect DMA gather   emb[p,:]   = embeds[cols[p], :]      (gpsimd)
  2. scale                 embs[p,:]  = emb[p,:] * vals[p]      (scalar)
  3. one-hot               oh[p,r]    = (rrow[p] == r)          (vector)
  4. matmul accumulate     psum[r,:] += oh.T @ embs             (tensor)
After a block's chunks, PSUM is copied to SBUF and DMA'd to the output
rows (contiguous -> plain DMA, no scatter).

The chunk schedule (chunks per block) is computed from the data on the
host and baked into the program; all 8 cores share one program, so the
per-block chunk count is the max over cores (~4% padding).
"""

import sys

import numpy as np

if "/opt/trn_rl_repo" not in sys.path:
    sys.path.insert(0, "/opt/trn_rl_repo")

N_NODES = 100000
D = 64
P = 128
N_CORES = 8


def _build_program(chunks_per_block, n_chunks, n_nodes, repeats=1):
    import concourse.bacc as bacc
    import concourse.bass as bass
    import concourse.tile as tile
    from concourse import mybir

    nodes_per_core = n_nodes // N_CORES
    n_blocks = len(chunks_per_block)

    nc = bacc.Bacc(
        "TRN2",
        target_bir_lowering=False,
        debug=False,
        num_devices=N_CORES,
    )
    embeds_t = nc.dram_tensor("embeds", [n_nodes, D], mybir.dt.float32, kind="ExternalInput")
    cols_t = nc.dram_tensor("cols_p", [P, n_chunks], mybir.dt.int32, kind="ExternalInput")
    vals_t = nc.dram_tensor("vals_p", [P, n_chunks], mybir.dt.float32, kind="ExternalInput")
    rrow_t = nc.dram_tensor("rrow_p", [P, n_chunks], mybir.dt.float32, kind="ExternalInput")
    iota_t = nc.dram_tensor("iota", [P, P], mybir.dt.float32, kind="ExternalInput")
    out_t = nc.dram_tensor("out", [n_blocks * P, D], mybir.dt.float32, kind="ExternalOutput")

    with tile.TileContext(nc) as tc:
        with (
            tc.tile_pool(name="static", bufs=1) as static_pool,
            tc.tile_pool(name="emb", bufs=8) as emb_pool,
            tc.tile_pool(name="sc", bufs=4) as sc_pool,
            tc.tile_pool(name="oh", bufs=4) as oh_pool,
            tc.tile_pool(name="outp", bufs=4) as out_pool,
            tc.tile_pool(name="psum", bufs=4, space="PSUM") as psum_pool,
        ):
            cols_sb = static_pool.tile([P, n_chunks], mybir.dt.int32)
            vals_sb = static_pool.tile([P, n_chunks], mybir.dt.float32)
            rrow_sb = static_pool.tile([P, n_chunks], mybir.dt.float32)
            iota_sb = static_pool.tile([P, P], mybir.dt.float32)
            nc.sync.dma_start(out=cols_sb[:], in_=cols_t[:])
            nc.sync.dma_start(out=vals_sb[:], in_=vals_t[:])
            nc.sync.dma_start(out=rrow_sb[:], in_=rrow_t[:])
            nc.sync.dma_start(out=iota_sb[:], in_=iota_t[:])

            for _rep in range(repeats):
              j = 0
              for b in range(n_blocks):
                nb = int(chunks_per_block[b])
                psum_tile = psum_pool.tile([P, D], dtype=mybir.dt.float32, space="PSUM")
                for t in range(nb):
                    emb_tile = emb_pool.tile([P, D], mybir.dt.float32)
                    nc.gpsimd.indirect_dma_start(
                        out=emb_tile[:],
                        out_offset=None,
                        in_=embeds_t[:],
                        in_offset=bass.IndirectOffsetOnAxis(
                            ap=cols_sb[:, j : j + 1], axis=0
                        ),
                    )
                    embs_tile = sc_pool.tile([P, D], mybir.dt.float32)
                    nc.scalar.activation(
                        out=embs_tile[:],
                        in_=emb_tile[:],
                        func=mybir.ActivationFunctionType.Copy,
                        scale=vals_sb[:, j : j + 1],
                    )
                    oh_tile = oh_pool.tile([P, P], mybir.dt.float32)
                    nc.vector.tensor_tensor(
                        out=oh_tile[:],
                        in0=rrow_sb[:, j : j + 1].to_broadcast([P, P]),
                        in1=iota_sb[:],
                        op=mybir.AluOpType.is_equal,
                    )
                    nc.tensor.matmul(
                        out=psum_tile[:],
                        lhsT=oh_tile[:],
                        rhs=embs_tile[:],
                        start=(t == 0),
                        stop=(t == nb - 1),
                    )
                    j += 1
                o_sb = out_pool.tile([P, D], mybir.dt.float32)
                nc.scalar.copy(out=o_sb[:], in_=psum_tile[:])
                nc.sync.dma_start(out=out_t[b * P : (b + 1) * P, :], in_=o_sb[:])
    nc.compile()
    return nc


def _kernel_impl(rows, cols, edge_vals, embeds, n_nodes, trace=False):
    from concourse.bass_utils import run_bass_kernel_spmd

    rows = np.asarray(rows).astype(np.int64)
    cs_all = np.asarray(cols).astype(np.int32)
    vs_all = np.asarray(edge_vals).astype(np.float32)
    embeds = np.ascontiguousarray(np.asarray(embeds), dtype=np.float32)

    nodes_per_core = n_nodes // N_CORES
    assert nodes_per_core * N_CORES == n_nodes
    n_blocks = (nodes_per_core + P - 1) // P

    order = np.argsort(rows, kind="stable")
    rs = rows[order]
    cs = cs_all[order]
    vs = vs_all[order]

    core_of_edge = rs // nodes_per_core
    blk_of_edge = (rs - core_of_edge * nodes_per_core) // P
    cnt = np.bincount(
        core_of_edge * n_blocks + blk_of_edge, minlength=N_CORES * n_blocks
    ).reshape(N_CORES, n_blocks)

    chunks_per_block = np.maximum(1, -(-cnt.max(axis=0) // P))  # ceil div
    n_chunks = int(chunks_per_block.sum())
    chunk_base = np.concatenate([[0], np.cumsum(chunks_per_block)])

    cols_p = np.zeros((N_CORES, n_chunks * P), np.int32)
    vals_p = np.zeros((N_CORES, n_chunks * P), np.float32)
    rrow_p = np.zeros((N_CORES, n_chunks * P), np.float32)
    core_edge_bounds = np.searchsorted(rs, np.arange(0, n_nodes + 1, nodes_per_core))
    for k in range(N_CORES):
        e0 = int(core_edge_bounds[k])
        for b in range(n_blocks):
            c = int(cnt[k, b])
            s = int(chunk_base[b]) * P
            cols_p[k, s : s + c] = cs[e0 : e0 + c]
            vals_p[k, s : s + c] = vs[e0 : e0 + c]
            rrow_p[k, s : s + c] = (
                rs[e0 : e0 + c] - k * nodes_per_core - b * P
            ).astype(np.float32)
            e0 += c

    # device layout: [P, n_chunks], partition p / chunk j <- edge j*P+p
    def dev(a, dt):
        return np.ascontiguousarray(
            a.reshape(N_CORES, n_chunks, P).transpose(0, 2, 1)
        ).astype(dt)

    cols_d = dev(cols_p, np.int32)
    vals_d = dev(vals_p, np.float32)
    rrow_d = dev(rrow_p, np.float32)
    iota = np.ascontiguousarray(
        np.tile(np.arange(P, dtype=np.float32), (P, 1))
    )

    nc = _build_program(chunks_per_block, n_chunks, n_nodes)
    in_maps = [
        {
            "embeds": embeds,
            "cols_p": cols_d[k],
            "vals_p": vals_d[k],
            "rrow_p": rrow_d[k],
            "iota": iota,
        }
        for k in range(N_CORES)
    ]
    global _LAST
    _LAST = (nc, in_maps)
    r = run_bass_kernel_spmd(nc, in_maps, list(range(N_CORES)), trace=trace)
    out = np.concatenate(
        [r.results[k]["out"][:nodes_per_core] for k in range(N_CORES)], axis=0
    ).astype(np.float32)
    if trace:
        return out, r
    return out


_LAST = None


def kernel(rows, cols, edge_vals, embeds):
    return _kernel_impl(rows, cols, edge_vals, embeds, N_NODES)



# revision 2
# speedup vs baseline: 1.0094x; 1.0094x over previous
"""GCN layer (SpMM) Bass kernel for 8 trn2 NeuronCores.

out[i] = sum_{e: rows[e]==i} edge_vals[e] * embeds[cols[e]]
N=100000 nodes, E=1000000 edges, D=64 features.

Strategy: host sorts edges by destination row and splits nodes into 8
contiguous ranges (12500 nodes/core) with disjoint outputs -> no
collectives. Per core, output rows are processed in blocks of 128; each
block's edges are padded to chunks of 128. Per chunk the device does:
  1. gather                emb[p,:]   = embeds[cols[p], :]      (SWDGE)
  2. scale (+cast bf16)    embs[p,:]  = emb[p,:] * vals[p]      (scalar)
  3. one-hot (bf16)        oh[p,r]    = (rrow[p] == r)          (vector)
  4. matmul accumulate     psum[r,:] += oh.T @ embs             (tensor)
After a block's chunks, PSUM is copied to SBUF and DMA'd to the output
rows (contiguous -> plain DMA, no scatter).

The gather uses SWDGE dma_gather (InstDMAGatherAnt): one instruction
fetches up to 32 chunks (4096 rows of 256B) spread over all 16 DMA
engines, instead of one HWDGE indirect DMA per chunk serialized on the
single qPoolDynamic queue. dma_gather indices are int16, so the embeds
table is split into 4 views of <=25000 rows; each block's edges are
grouped by col-quartile on the host and chunk-padded per group. Chunk
slots are laid out group-major so each group's gathers cover long
consecutive token runs.

The chunk schedule is computed from the data on the host and baked into
the program; all 8 cores share one program, so per-(block,group) chunk
counts are the max over cores.
"""

import sys

import numpy as np

if "/opt/trn_rl_repo" not in sys.path:
    sys.path.insert(0, "/opt/trn_rl_repo")

N_NODES = 100000
D = 64
P = 128
N_CORES = 8
N_GROUPS = 4  # embeds views (int16 gather idx => <=32768 rows per view)
G_CHUNKS = 32  # chunks per dma_gather instruction (4096 rows)


def _schedule(chunks_bg):
    """Group-major slot layout. Returns (Tg, gbase, off_bg, n_chunks)."""
    n_blocks = chunks_bg.shape[0]
    Tg = chunks_bg.sum(axis=0).astype(np.int64)  # chunks per group
    gbase = np.concatenate([[0], np.cumsum(Tg)]).astype(np.int64)
    off_bg = np.zeros((n_blocks, N_GROUPS), np.int64)
    for g in range(N_GROUPS):
        off_bg[:, g] = gbase[g] + np.concatenate(
            [[0], np.cumsum(chunks_bg[:-1, g])]
        )
    return Tg, gbase, int(chunks_bg.sum()), off_bg


def _build_program(chunks_bg, n_nodes):
    import concourse.bacc as bacc
    import concourse.tile as tile
    from concourse import mybir

    n_blocks = chunks_bg.shape[0]
    group_size = -(-n_nodes // N_GROUPS)
    Tg, gbase, n_chunks, off_bg = _schedule(chunks_bg)

    nc = bacc.Bacc(
        "TRN2",
        target_bir_lowering=False,
        debug=False,
        num_devices=N_CORES,
        num_swdge_queues=4,
    )
    embeds_t = nc.dram_tensor(
        "embeds", [n_nodes, D], mybir.dt.float32, kind="ExternalInput"
    )
    idx_t = nc.dram_tensor(
        "idx_p", [P, n_chunks * (P // 16)], mybir.dt.int16, kind="ExternalInput"
    )
    vals_t = nc.dram_tensor("vals_p", [P, n_chunks], mybir.dt.float32, kind="ExternalInput")
    rrow_t = nc.dram_tensor("rrow_p", [P, n_chunks], mybir.dt.bfloat16, kind="ExternalInput")
    iota_t = nc.dram_tensor("iota", [P, P], mybir.dt.bfloat16, kind="ExternalInput")
    out_t = nc.dram_tensor(
        "out", [n_blocks * P, D], mybir.dt.float32, kind="ExternalOutput"
    )

    with tile.TileContext(nc) as tc:
        with (
            tc.tile_pool(name="static", bufs=1) as static_pool,
            tc.tile_pool(name="gat0", bufs=2) as gp0,
            tc.tile_pool(name="gat1", bufs=2) as gp1,
            tc.tile_pool(name="gat2", bufs=2) as gp2,
            tc.tile_pool(name="gat3", bufs=2) as gp3,
            tc.tile_pool(name="sc", bufs=4) as sc_pool,
            tc.tile_pool(name="oh", bufs=4) as oh_pool,
            tc.tile_pool(name="outp", bufs=4) as out_pool,
            tc.tile_pool(name="psum", bufs=4, space="PSUM") as psum_pool,
        ):
            idx_sb = static_pool.tile([P, n_chunks * (P // 16)], mybir.dt.int16)
            vals_sb = static_pool.tile([P, n_chunks], mybir.dt.float32)
            rrow_sb = static_pool.tile([P, n_chunks], mybir.dt.bfloat16)
            iota_sb = static_pool.tile([P, P], mybir.dt.bfloat16)
            nc.sync.dma_start(out=idx_sb[:], in_=idx_t[:])
            nc.sync.dma_start(out=vals_sb[:], in_=vals_t[:])
            nc.sync.dma_start(out=rrow_sb[:], in_=rrow_t[:])
            nc.sync.dma_start(out=iota_sb[:], in_=iota_t[:])

            gat_pools = [gp0, gp1, gp2, gp3]
            gtiles = [[] for _ in range(N_GROUPS)]  # instruction tiles per group
            next_instr = [0] * N_GROUPS
            qrr = [0]  # SWDGE queue round-robin

            def ensure(g, upto_gc):
                # emit gathers for group g until group-chunks [0, upto_gc) covered
                while next_instr[g] * G_CHUNKS < upto_gc:
                    j = next_instr[g]
                    g0 = j * G_CHUNKS
                    n_i = int(min(G_CHUNKS, Tg[g] - g0))
                    tl = gat_pools[g].tile([P, G_CHUNKS, D], mybir.dt.float32)
                    slot0 = int(gbase[g] + g0)
                    r0 = g * group_size
                    r1 = min((g + 1) * group_size, n_nodes)
                    nc.gpsimd.dma_gather(
                        tl[:, :n_i, :],
                        embeds_t[r0:r1, :],
                        idx_sb[:, slot0 * (P // 16) : (slot0 + n_i) * (P // 16)],
                        n_i * P,
                        n_i * P,
                        D,
                        queue_num=qrr[0],
                    )
                    qrr[0] = (qrr[0] + 1) % 4
                    gtiles[g].append(tl)
                    next_instr[g] += 1

            for b in range(n_blocks):
                tot_b = int(chunks_bg[b].sum())
                psum_tile = psum_pool.tile([P, D], dtype=mybir.dt.float32, space="PSUM")
                t = 0
                for g in range(N_GROUPS):
                    cbg = int(chunks_bg[b, g])
                    if cbg == 0:
                        continue
                    gc0 = int(off_bg[b, g] - gbase[g])
                    ensure(g, gc0 + cbg)
                    for c in range(cbg):
                        gc = gc0 + c
                        slot = int(off_bg[b, g] + c)
                        tl = gtiles[g][gc // G_CHUNKS]
                        o = gc % G_CHUNKS
                        embs_tile = sc_pool.tile([P, D], mybir.dt.bfloat16)
                        nc.scalar.activation(
                            out=embs_tile[:],
                            in_=tl[:, o : o + 1, :],
                            func=mybir.ActivationFunctionType.Copy,
                            scale=vals_sb[:, slot : slot + 1],
                        )
                        oh_tile = oh_pool.tile([P, P], mybir.dt.bfloat16)
                        nc.vector.tensor_tensor(
                            out=oh_tile[:],
                            in0=rrow_sb[:, slot : slot + 1].to_broadcast([P, P]),
                            in1=iota_sb[:],
                            op=mybir.AluOpType.is_equal,
                        )
                        nc.tensor.matmul(
                            out=psum_tile[:],
                            lhsT=oh_tile[:],
                            rhs=embs_tile[:],
                            start=(t == 0),
                            stop=(t == tot_b - 1),
                        )
                        t += 1
                o_sb = out_pool.tile([P, D], mybir.dt.float32)
                nc.scalar.copy(out=o_sb[:], in_=psum_tile[:])
                nc.sync.dma_start(out=out_t[b * P : (b + 1) * P, :], in_=o_sb[:])
    nc.compile()
    return nc


def _kernel_impl(rows, cols, edge_vals, embeds, n_nodes, trace=False):
    import ml_dtypes

    from concourse.bass_utils import run_bass_kernel_spmd

    rows = np.asarray(rows).astype(np.int64)
    cs_all = np.asarray(cols).astype(np.int32)
    vs_all = np.asarray(edge_vals).astype(np.float32)
    embeds = np.ascontiguousarray(np.asarray(embeds), dtype=np.float32)

    npc = n_nodes // N_CORES
    assert npc * N_CORES == n_nodes
    n_blocks = (npc + P - 1) // P
    group_size = -(-n_nodes // N_GROUPS)
    assert group_size <= 32767

    core = rows // npc
    blk = (rows % npc) // P
    rrow = (rows % npc) % P
    grp = cs_all // group_size
    bkey = ((core * n_blocks + blk) * N_GROUPS + grp).astype(np.int64)
    order = np.argsort(bkey, kind="stable")
    bkey_s = bkey[order]
    cs_s = cs_all[order]
    vs_s = vs_all[order]
    rrow_s = rrow[order]

    n_seg = N_CORES * n_blocks * N_GROUPS
    cnt = np.bincount(bkey_s, minlength=n_seg).reshape(N_CORES, n_blocks, N_GROUPS)
    chunks_bg = -(-cnt.max(axis=0) // P)  # [n_blocks, N_GROUPS]
    forced = chunks_bg.sum(axis=1) == 0
    chunks_bg[forced, 0] = 1
    Tg, gbase, n_chunks, off_bg = _schedule(chunks_bg)

    # position of each edge inside its (core, block, group) segment
    seg_start = np.zeros(n_seg + 1, np.int64)
    np.cumsum(cnt.ravel(), out=seg_start[1:])
    pos_in_seg = np.arange(len(rows), dtype=np.int64) - seg_start[bkey_s]

    b_s = (bkey_s // N_GROUPS) % n_blocks
    g_s = bkey_s % N_GROUPS
    k_s = bkey_s // (n_blocks * N_GROUPS)
    slot_s = off_bg[b_s, g_s] + pos_in_seg // P  # global chunk slot
    part_s = pos_in_seg % P

    vals_p = np.zeros((N_CORES, n_chunks, P), np.float32)
    rrow_p = np.zeros((N_CORES, n_chunks, P), np.float32)
    tok_p = np.zeros((N_CORES, n_chunks, P), np.int16)
    vals_p[k_s, slot_s, part_s] = vs_s
    rrow_p[k_s, slot_s, part_s] = rrow_s
    tok_p[k_s, slot_s, part_s] = (cs_s - g_s * group_size).astype(np.int16)

    # device layouts
    vals_d = np.ascontiguousarray(vals_p.transpose(0, 2, 1))  # [8, P, n_chunks]
    rrow_d = np.ascontiguousarray(rrow_p.transpose(0, 2, 1)).astype(ml_dtypes.bfloat16)
    # idx wrap: token t at partition t%16, col t//16; replicate to 128 partitions
    idx_d = np.ascontiguousarray(
        np.tile(
            tok_p.reshape(N_CORES, n_chunks * P // 16, 16).transpose(0, 2, 1),
            (1, 8, 1),
        )
    )  # [8, 128, n_chunks*8]
    iota = np.tile(np.arange(P, dtype=np.float32), (P, 1)).astype(ml_dtypes.bfloat16)

    nc = _build_program(chunks_bg, n_nodes)
    in_maps = [
        {
            "embeds": embeds,
            "idx_p": idx_d[k],
            "vals_p": vals_d[k],
            "rrow_p": rrow_d[k],
            "iota": iota,
        }
        for k in range(N_CORES)
    ]
    global _LAST
    _LAST = (nc, in_maps)
    r = run_bass_kernel_spmd(nc, in_maps, list(range(N_CORES)), trace=trace)
    out = np.concatenate(
        [r.results[k]["out"][:npc] for k in range(N_CORES)], axis=0
    ).astype(np.float32)
    if trace:
        return out, r
    return out


_LAST = None


def kernel(rows, cols, edge_vals, embeds):
    return _kernel_impl(rows, cols, edge_vals, embeds, N_NODES)
